# revision 15
# baseline (speedup 1.0000x reference)
"""Trainium2 Bass kernel for the quantized Conv2d (nn_Conv2d_47356309405843).

Reference semantics (qtorch-style float_quantize == IEEE saturating RNE
casts, verified elementwise):
  x_q = fp8e5m2(x); w_q = fp8e5m2(weight)
  y = 0
  for g in 8 channel groups (8 ch each):           # 72 serial steps
    for each of 9 taps (ih, iw):
      y = fp16(y + einsum(x_q[group, shifted], w_q[group, tap]) + bias)

This kernel reproduces the CPU-semantics reference BIT-EXACTLY (for the
given zero bias): fp8e5m2 products are exact in the PE's fp22/fp32
accumulation, and the DVE computes psum_f32 + y_f16 in fp32 and rounds
once to fp16 (RNE) on write — identical to the reference's fp32 add +
fp16 requantize at every one of the 72 steps.

Structure (per core, batch-sharded 2 images/core over 8 cores):
  - host: cast inputs to fp8e5m2, zero-pad x, replicate x/w at partition
    bases 0/32/64/96 (PE row-group strips)
  - PE: per (step, 28-row chunk): 4 matmuls [K=8, M=128, N=392], each into
    its own PSUM bank of a [128,4,512] tile; tile_position cycles the four
    32-row PE strips across steps so adjacent steps' matmuls run
    concurrently in the systolic array (K=8 leaves the array mostly idle
    otherwise)
  - DVE (the bottleneck, ~0.5 cyc/elem): one tensor_add per (step, chunk)
    drains PSUM and requantizes: y16 = fp16(psum + y16)
  - ACT: final fp16 -> fp32 upconvert, then DMA out
"""

import numpy as np
import ml_dtypes
from contextlib import ExitStack

import concourse.bass as bass
import concourse.tile as tile
from concourse import bacc, mybir
from concourse.bass_utils import run_bass_kernel_spmd

# problem constants (hardcoded per contract)
B, C_IN, H, W = 16, 64, 56, 56
C_OUT, K, C_TC, PAD = 128, 3, 8, 1
N_CORES = 8
B_PC = B // N_CORES          # images per core
HP, WP = H + 2 * PAD, W + 2 * PAD            # 58, 58
SPI = H * W                                   # spatial per image 3136
NG = C_IN // C_TC                             # 8 channel groups
NSTEP = NG * K * K                            # 72 accumulation steps

# chunking of the output spatial dim (rows of 56)
ROWS_PER_MM = 7                               # 7*56 = 392 <= 512 (one PSUM bank)
MM_PER_CHUNK = 4                              # chunk = 28 rows = 1568 cols
ROWS_PER_CHUNK = ROWS_PER_MM * MM_PER_CHUNK   # 28
CHUNKS_PER_IMG = H // ROWS_PER_CHUNK          # 2
NCHUNK = CHUNKS_PER_IMG * B_PC                # 4 chunks per core
FCH = ROWS_PER_CHUNK * W                      # 1568 cols per chunk
FMM = ROWS_PER_MM * W                         # 392 cols per matmul

# how many of the chunks go through the ACT(copy)->DVE(fp16 add) path.
# 0 = pure DVE fp32 drain (exact single-rounding semantics everywhere)
N_ACT_CHUNKS = 0

# rotate PE row-group per (step,chunk) instead of per step
STRIP_PER_CHUNK = True

_COMPILED = {}


def _build(repeats=1, n_act=N_ACT_CHUNKS, has_bias=False, strips=True):
    nc = bacc.Bacc("TRN2", target_bir_lowering=False, debug=False,
                   num_devices=N_CORES)
    xin = nc.dram_tensor("xin", [C_TC, NG * B_PC * HP * WP], mybir.dt.float8e5,
                         kind="ExternalInput").ap()
    win = nc.dram_tensor("win", [C_TC, NSTEP * C_OUT], mybir.dt.float8e5,
                         kind="ExternalInput").ap()
    bin_ = (nc.dram_tensor("bin", [C_OUT, 1], mybir.dt.float32,
                           kind="ExternalInput").ap() if has_bias else None)
    yout = nc.dram_tensor("yout", [C_OUT, B_PC * SPI], mybir.dt.float32,
                          kind="ExternalOutput").ap()

    with tile.TileContext(nc) as tc:
        with ExitStack() as ctx:
            _emit(tc, ctx, xin, win, yout, repeats=repeats, n_act=n_act,
                  bin_=bin_, strips=strips)
    nc.compile()
    return nc


def _emit(tc, ctx, xin, win, yout, repeats=1, n_act=N_ACT_CHUNKS,
          bin_=None, strips=True):
    nc = tc.nc
    f8, f16, f32 = mybir.dt.float8e5, mybir.dt.float16, mybir.dt.float32

    singles = ctx.enter_context(tc.tile_pool(name="singles", bufs=1))
    psum_pool = ctx.enter_context(tc.tile_pool(name="ps", bufs=2, space="PSUM"))
    p16_pool = ctx.enter_context(tc.tile_pool(name="p16", bufs=6))
    out_pool = ctx.enter_context(tc.tile_pool(name="outs", bufs=3))

    # x: [8 ch, group, img, row, col] ; w: [8 ch, step, cout]
    # replicated at partition bases 0/32/64/96 so consecutive steps can run
    # in different PE row groups (concurrent 32-row tiles).
    nstrip = 4 if strips else 1
    xg = singles.tile([128, NG, B_PC, HP, WP], f8)
    wt = singles.tile([128, NSTEP, C_OUT], f8)
    for s4 in range(nstrip):
        nc.sync.dma_start(xg[32 * s4:32 * s4 + C_TC],
                          xin.rearrange("c (g i r q) -> c g i r q",
                                        g=NG, i=B_PC, r=HP))
        nc.sync.dma_start(wt[32 * s4:32 * s4 + C_TC],
                          win.rearrange("c (s o) -> c s o", s=NSTEP))
    bias_sb = None
    if bin_ is not None:
        bias_sb = singles.tile([128, 1], f32)
        nc.sync.dma_start(bias_sb[:], bin_[:])

    # fp16 accumulator, zero-initialized
    y16 = singles.tile([128, NCHUNK, FCH], f16)

    act_chunks = set(range(NCHUNK - n_act, NCHUNK))

    for _rep in range(repeats):
        _emit_once(tc, nc, xg, wt, y16, yout, act_chunks, p16_pool, psum_pool,
                   out_pool, bias_sb, nstrip)


def _emit_once(tc, nc, xg, wt, y16, yout, act_chunks, p16_pool, psum_pool,
               out_pool, bias_sb=None, nstrip=1):
    f16, f32 = mybir.dt.float16, mybir.dt.float32
    # no memset: step 0 writes y16 directly (y = fp16(p [+ bias]))

    for s in range(NSTEP):
        g, t = divmod(s, K * K)
        ih, iw = divmod(t, K)
        for c in range(NCHUNK):
            sp = 32 * (((s * NCHUNK + c) if STRIP_PER_CHUNK else s) % nstrip)
            img, cr = divmod(c, CHUNKS_PER_IMG)
            r0 = cr * ROWS_PER_CHUNK
            pt = psum_pool.tile([128, MM_PER_CHUNK, 512], f32, tag="ps")
            for m in range(MM_PER_CHUNK):
                rr = r0 + m * ROWS_PER_MM + ih
                nc.tensor.matmul(
                    pt[:, m, :FMM],
                    wt[sp:sp + C_TC, s, :],
                    xg[sp:sp + C_TC, g, img, rr:rr + ROWS_PER_MM, iw:iw + W],
                    start=True, stop=True,
                    tile_position=(sp, 0),
                )
            ysl = y16[:, c, :].rearrange("p (a b) -> p a b", a=MM_PER_CHUNK)
            if s == 0:
                # first step initializes y: y = fp16(p [+ bias])
                if bias_sb is not None:
                    nc.vector.tensor_scalar_add(ysl, pt[:, :, :FMM],
                                                bias_sb[:, 0:1])
                else:
                    nc.vector.tensor_copy(ysl, pt[:, :, :FMM])
            elif c in act_chunks:
                # ACT: round partial to fp16 in SBUF; DVE: fp16 add (2x mode)
                p16 = p16_pool.tile([128, MM_PER_CHUNK, FMM], f16, tag="p16")
                nc.scalar.copy(p16[:], pt[:, :, :FMM])
                nc.vector.tensor_add(ysl, p16[:], ysl)
            elif bias_sb is not None:
                nc.vector.scalar_tensor_tensor(
                    ysl, pt[:, :, :FMM], bias_sb[:, 0:1], ysl,
                    op0=mybir.AluOpType.add, op1=mybir.AluOpType.add)
            else:
                # exact: fp32 add from PSUM, fp16 RNE on write
                nc.vector.tensor_add(ysl, pt[:, :, :FMM], ysl)

    # upconvert fp16 -> fp32 and store
    for c in range(NCHUNK):
        y32 = out_pool.tile([128, FCH], f32, tag="y32")
        nc.scalar.copy(y32[:], y16[:, c, :])
        nc.sync.dma_start(yout[:, c * FCH:(c + 1) * FCH], y32[:])


def _prep_inputs(x, weight):
    """Host-side quantize + layout. Returns per-core input maps."""
    f8 = ml_dtypes.float8_e5m2
    xq = x.astype(f8)
    wq = weight.astype(f8)                     # [C_OUT, C_IN, K, K]
    xp = np.zeros((B, C_IN, HP, WP), f8)
    xp[:, :, PAD:PAD + H, PAD:PAD + W] = xq

    # win[c, (g*9+t)*128 + o] = wq[o, 8g+c, t//3, t%3]
    wr = wq.reshape(C_OUT, NG, C_TC, K * K)          # o, g, c, t
    wr = wr.transpose(2, 1, 3, 0)                    # c, g, t, o
    win = np.ascontiguousarray(wr.reshape(C_TC, NSTEP * C_OUT))

    in_maps = []
    for core in range(N_CORES):
        xs = xp[core * B_PC:(core + 1) * B_PC]       # [2, 64, 58, 58]
        xs = xs.transpose(1, 0, 2, 3)                # [64, 2, 58, 58]
        xs = xs.reshape(NG, C_TC, B_PC, HP, WP).transpose(1, 0, 2, 3, 4)
        xin = np.ascontiguousarray(xs.reshape(C_TC, NG * B_PC * HP * WP))
        in_maps.append({"xin": xin, "win": win})
    return in_maps


def kernel(x, weight, bias, _trace=False):
    x = np.asarray(x, np.float32)
    weight = np.asarray(weight, np.float32)
    bias = np.asarray(bias, np.float32)
    has_bias = bool(np.any(bias))

    key = ("nc", has_bias)
    if key not in _COMPILED:
        _COMPILED[key] = _build(has_bias=has_bias)
    nc = _COMPILED[key]

    in_maps = _prep_inputs(x, weight)
    if has_bias:
        for m in in_maps:
            m["bin"] = np.ascontiguousarray(bias.reshape(C_OUT, 1))
    res = run_bass_kernel_spmd(nc, in_maps, list(range(N_CORES)),
                               trace=_trace)

    y = np.empty((B, C_OUT, H, W), np.float32)
    for core in range(N_CORES):
        yo = res.results[core]["yout"]               # [128, B_PC*SPI]
        yo = yo.reshape(C_OUT, B_PC, CHUNKS_PER_IMG, ROWS_PER_CHUNK, W)
        # chunk-major free layout: [img, chunk, rows, col] -> [img, 56, 56]
        yo = yo.transpose(1, 0, 2, 3, 4).reshape(B_PC, C_OUT, H, W)
        y[core * B_PC:(core + 1) * B_PC] = yo
    if _trace:
        return y, res
    return y
